# revision 18
# baseline (speedup 1.0000x reference)
"""Segment min/max pooling (JunctionPool) on 8 Trainium2 NeuronCores.

Full inputs:
    edge_features  [2097152, 64] float32
    cell_0_bounds  [524288, 2]   int32   (begin, end) per junction, contiguous
Output:
    [524288, 128] float32 = concat([segment_min, segment_max], axis=1)

Strategy (matches the reference's searchsorted-on-ends semantics):
  * Segments are contiguous ranges of edges sorted by junction; segment j is
    [ends[j-1], ends[j]).  The generated bounds repeat lengths [1, 3, 4, 8]
    (period: 4 junctions == 16 edges).
  * Shard both edges and junctions into 8 contiguous, period-aligned ranges;
    each core reduces its own ranges - no cross-core communication.
  * The rel-err budget (2e-2) admits 16-bit compute: the host rounds edge
    features to bfloat16 (round-to-nearest-even), the device reduces in
    bf16 (min/max comparisons are exact), writes bf16, and the host
    upcasts the gathered output to float32.  This halves HBM traffic
    (the memory roofline) and doubles DVE throughput (2x perf mode).
  * The length-1 segment of each period (junction 0) satisfies
    min == max == the raw edge value, so the device neither computes nor
    stores it; the host fills those output rows directly from the f32
    input (exactly).  Device output is junctions 1..3 per period only:
    25% fewer output bytes over the memory-bound DMA path.
  * On-chip layout: each SBUF partition holds whole 16-edge periods, so the
    HBM->SBUF loads and SBUF->HBM stores are fully dense, and the ragged
    reduction becomes static strided tensor_tensor ops per tile
    (3 length-classes x {min, max}) on the vector engine.
  * The host verifies the [1,3,4,8] pattern from the actual bounds tensor at
    run time; anything else falls back to a generic host reduction.
"""

import sys
import types

if "/opt/trn_rl_repo" not in sys.path:
    sys.path.insert(0, "/opt/trn_rl_repo")

import numpy as np


def _ensure_axon_hooks_module():
    """bass_utils imports antenv.axon_hooks when BASS_TRACE=1; some images
    lack that module. Provide a minimal stand-in so tracing degrades
    gracefully instead of crashing."""
    try:
        import antenv.axon_hooks  # noqa: F401
        return
    except ImportError:
        pass
    try:
        import antenv
    except ImportError:
        return
    mod = types.ModuleType("antenv.axon_hooks")
    mod._hook = None

    def set_axon_ntff_profile_hook(h):
        mod._hook = h

    def get_axon_ntff_profile_hook():
        return mod._hook

    mod.set_axon_ntff_profile_hook = set_axon_ntff_profile_hook
    mod.get_axon_ntff_profile_hook = get_axon_ntff_profile_hook
    sys.modules["antenv.axon_hooks"] = mod
    antenv.axon_hooks = mod


_ensure_axon_hooks_module()

E_TOTAL = 2097152
C = 64
J_TOTAL = 524288
N_CORES = 8
PATTERN = (1, 3, 4, 8)  # segment lengths within one period
PERIOD_EDGES = 16
PERIOD_JUNCS = 4

E_LOC = E_TOTAL // N_CORES  # 262144 edges per core
J_LOC = J_TOTAL // N_CORES  # 65536 junctions per core
Q_LOC = J_LOC // PERIOD_JUNCS  # 16384 periods per core
R_OUT = 3  # device stores junctions 1..3 per period (junction 0 is host-filled)

# Periods-per-partition for each tile. Uniform big tiles keep the DVE op
# count low; the tapered tail shrinks the final (unoverlappable) output DMA.
# Sums to 128 (= total periods per partition per core).
G_LIST = (8, 16, 16, 16, 16, 16, 16, 12, 8, 4)
G_MAX = max(G_LIST)

_COMPILED = None
LAST_RESULTS = None  # BassKernelResults of the most recent device run


def _build_program():
    import concourse.bacc as bacc
    import concourse.mybir as mybir
    from concourse.tile import TileContext

    MIN = mybir.AluOpType.min
    MAX = mybir.AluOpType.max
    BF16 = mybir.dt.bfloat16

    nc = bacc.Bacc()
    edges = nc.declare_dram_parameter("edges", [E_LOC, C], BF16, isOutput=False)
    out = nc.declare_dram_parameter(
        "out", [Q_LOC * R_OUT, 2 * C], BF16, isOutput=True
    )

    with TileContext(nc) as tc:
        with tc.tile_pool(name="iin", bufs=3) as pin, tc.tile_pool(
            name="iout", bufs=3
        ) as pout, tc.tile_pool(name="tmp", bufs=2) as tmp:
            edge_row = 0
            out_row = 0
            for g in G_LIST:
                # Per-tile views: partition p holds g whole 16-edge periods.
                n_in = 128 * g * PERIOD_EDGES
                n_out = 128 * g * R_OUT
                in_view = edges[edge_row : edge_row + n_in, :].rearrange(
                    "(p j) c -> p (j c)", p=128
                )
                out_view = out[out_row : out_row + n_out, :].rearrange(
                    "(p r) c -> p (r c)", p=128
                )
                edge_row += n_in
                out_row += n_out

                tile = pin.tile([128, g * PERIOD_EDGES * C], BF16, tag="itile")
                nc.sync.dma_start(out=tile[:], in_=in_view)
                otile = pout.tile([128, g * R_OUT * 2 * C], BF16, tag="otile")
                # v[p, g, x]: x = flat elements of one 16-edge period (1024)
                # edge e occupies x[e*64:(e+1)*64]; junction layout per period:
                # j0 = edge 0, j1 = edges 1..3, j2 = edges 4..7, j3 = 8..15.
                v = tile.rearrange("p (g x) -> p g x", g=g)
                # w[p, g, r, c]: junction r+1 of period g; c 0:64 = min,
                # 64:128 = max
                w = otile.rearrange(
                    "p (g r c) -> p g r c", g=g, r=R_OUT, c=2 * C
                )

                def tt(op, o, a, b):
                    nc.vector.tensor_tensor(out=o, in0=a, in1=b, op=op)

                for op, lo in ((MIN, 0), (MAX, C)):
                    sl = slice(lo, lo + C)
                    # Level 1, one op: pair edges (2k, 2k+1) for k=1..7 - every
                    # pair lies within a single segment (j1:{2,3}, j2:{4,5},
                    # {6,7}, j3:{8,9}..{14,15}).  m[k-1] = op(e_2k, e_2k+1).
                    tm = tmp.tile([128, g * 7 * C], BF16, tag="tm")
                    m = tm.rearrange("p (g x) -> p g x", g=g)
                    pairs = v[:, :, 2 * C : 16 * C].rearrange(
                        "p g (k c) -> p g k c", k=7
                    )
                    tt(op, m[:], pairs[:, :, :, 0:C], pairs[:, :, :, C : 2 * C])
                    # Level 2a, one op: pairs among m: (m1,m2)->j2, (m3,m4)->n0,
                    # (m5,m6)->n1   [q = [j2, n0, n1], contiguous]
                    tq = tmp.tile([128, g * 3 * C], BF16, tag="tq")
                    q = tq.rearrange("p (g x) -> p g x", g=g)
                    mp = m[:, :, C : 7 * C].rearrange("p g (k c) -> p g k c", k=3)
                    tt(op, q[:], mp[:, :, :, 0:C], mp[:, :, :, C : 2 * C])
                    # Level 2b: junction 1 (len-3) = op(m0, e1)
                    tt(op, w[:, :, 0, sl], m[:, :, 0:C], v[:, :, C : 2 * C])
                    # Level 3: junction 3 (len-8) = op(n0, n1)
                    tt(op, w[:, :, 2, sl], q[:, :, C : 2 * C], q[:, :, 2 * C : 3 * C])
                    # junction 2 (len-4) is q[0]; the idle scalar engine
                    # moves it into the output tile.
                    nc.scalar.copy(out=w[:, :, 1, sl], in_=q[:, :, 0:C])
                # Output DMAs go out via SWDGE (GpSimd): the SP HWDGE ring then
                # carries only input loads and never stalls behind an output's
                # wait-for-compute, so the input stream (which gates the DVE)
                # flows continuously.  (DVE here only runs tensor_tensor =
                # 2x_1P single-port mode, which never locks GpSimd out of the
                # shared SBUF port pair.)
                nc.gpsimd.dma_start(out=out_view, in_=otile[:])

    nc.compile()
    return nc


def _get_program():
    global _COMPILED
    if _COMPILED is None:
        _COMPILED = _build_program()
    return _COMPILED


def _to_bf16(x: np.ndarray):
    """f32 -> bf16 with round-to-nearest-even, as an ml_dtypes.bfloat16 view."""
    import ml_dtypes

    u = np.ascontiguousarray(x, dtype=np.float32).view(np.uint32)
    bias = ((u >> 16) & 1) + np.uint32(0x7FFF)
    return ((u + bias) >> 16).astype(np.uint16).view(ml_dtypes.bfloat16)


def _from_bf16(x: np.ndarray) -> np.ndarray:
    """bf16 (any 2-byte view) -> f32, exact."""
    u = np.ascontiguousarray(x).view(np.uint16).astype(np.uint32) << 16
    return u.view(np.float32)


def _pattern_matches(bounds: np.ndarray) -> bool:
    if bounds.shape != (J_TOTAL, 2):
        return False
    ends = bounds[:, 1].astype(np.int64)
    lengths = np.diff(ends, prepend=0)
    expect = np.tile(np.asarray(PATTERN, np.int64), J_TOTAL // PERIOD_JUNCS)
    return bool(ends[-1] == E_TOTAL and np.array_equal(lengths, expect))


def _fallback_host(edge_features: np.ndarray, bounds: np.ndarray) -> np.ndarray:
    # Generic reduction matching the reference's searchsorted-on-ends
    # semantics, including empty segments (+inf/-inf identities).
    ends = bounds[:, 1].astype(np.int64)
    J = bounds.shape[0]
    E = edge_features.shape[0]
    starts = np.concatenate([[0], ends[:-1]])
    starts = np.clip(starts, 0, E)
    ends_c = np.clip(ends, 0, E)
    mins = np.full((J, edge_features.shape[1]), np.inf, np.float32)
    maxs = np.full((J, edge_features.shape[1]), -np.inf, np.float32)
    for j in range(J):
        s, e = starts[j], ends_c[j]
        if e > s:
            seg = edge_features[s:e]
            mins[j] = seg.min(axis=0)
            maxs[j] = seg.max(axis=0)
    return np.concatenate([mins, maxs], axis=1)


def kernel(edge_features, cell_0_bounds) -> np.ndarray:
    global LAST_RESULTS
    edge_features = np.ascontiguousarray(np.asarray(edge_features, dtype=np.float32))
    cell_0_bounds = np.asarray(cell_0_bounds, dtype=np.int32)

    if edge_features.shape != (E_TOTAL, C) or not _pattern_matches(cell_0_bounds):
        return _fallback_host(edge_features, cell_0_bounds)

    from concourse.bass_utils import run_bass_kernel_spmd

    nc = _get_program()
    edges_bf16 = _to_bf16(edge_features)
    in_maps = [
        {"edges": edges_bf16[i * E_LOC : (i + 1) * E_LOC]} for i in range(N_CORES)
    ]
    res = run_bass_kernel_spmd(nc, in_maps, core_ids=list(range(N_CORES)))
    LAST_RESULTS = res

    full = np.empty((J_TOTAL, 2 * C), dtype=np.float32)
    # junction 0 of each period: min == max == edge 0 of the period, exact f32
    j0 = edge_features[0::PERIOD_EDGES]
    full[0::PERIOD_JUNCS, 0:C] = j0
    full[0::PERIOD_JUNCS, C:] = j0
    quads = full.reshape(J_TOTAL // PERIOD_JUNCS, PERIOD_JUNCS, 2 * C)
    for i, r in enumerate(res.results):
        dev = _from_bf16(r["out"]).reshape(Q_LOC, R_OUT, 2 * C)
        quads[i * Q_LOC : (i + 1) * Q_LOC, 1:, :] = dev
    return full


# revision 20
# speedup vs baseline: 1.1172x; 1.1172x over previous
"""Segment min/max pooling (JunctionPool) on 8 Trainium2 NeuronCores.

Full inputs:
    edge_features  [2097152, 64] float32
    cell_0_bounds  [524288, 2]   int32   (begin, end) per junction, contiguous
Output:
    [524288, 128] float32 = concat([segment_min, segment_max], axis=1)

Strategy (matches the reference's searchsorted-on-ends semantics):
  * Segments are contiguous ranges of edges sorted by junction; segment j is
    [ends[j-1], ends[j]).  The generated bounds repeat lengths [1, 3, 4, 8]
    (period: 4 junctions == 16 edges).
  * Shard both edges and junctions into 8 contiguous, period-aligned ranges;
    each core reduces its own ranges - no cross-core communication.
  * The rel-err budget (2e-2) admits 16-bit compute: the host rounds edge
    features to bfloat16 (round-to-nearest-even), the device reduces in
    bf16 (min/max comparisons are exact), writes bf16, and the host
    upcasts the gathered output to float32.  This halves HBM traffic
    (the memory roofline) and doubles DVE throughput (2x perf mode).
  * The length-1 segment of each period (junction 0) satisfies
    min == max == the raw edge value, so the device neither computes nor
    stores it; the host fills those output rows directly from the f32
    input (exactly).  Device output is junctions 1..3 per period only:
    25% fewer output bytes over the memory-bound DMA path.
  * On-chip layout: each SBUF partition holds whole 16-edge periods, so the
    HBM->SBUF loads and SBUF->HBM stores are fully dense, and the ragged
    reduction becomes 4 static strided tensor_tensor ops per {min, max} on
    the vector engine (even/odd edge pairing merges all three length
    classes' first two tree levels into single large ops); the idle scalar
    engine moves the len-4 result into the output tile.
  * Input loads ride the SP HWDGE ring; output stores go out via SWDGE
    (GpSimd), so the input stream that gates the DVE never queues behind an
    output's wait-for-compute.
  * The host verifies the [1,3,4,8] pattern from the actual bounds tensor at
    run time; anything else falls back to a generic host reduction.
"""

import sys
import types

if "/opt/trn_rl_repo" not in sys.path:
    sys.path.insert(0, "/opt/trn_rl_repo")

import numpy as np


def _ensure_axon_hooks_module():
    """bass_utils imports antenv.axon_hooks when BASS_TRACE=1; some images
    lack that module. Provide a minimal stand-in so tracing degrades
    gracefully instead of crashing."""
    try:
        import antenv.axon_hooks  # noqa: F401
        return
    except ImportError:
        pass
    try:
        import antenv
    except ImportError:
        return
    mod = types.ModuleType("antenv.axon_hooks")
    mod._hook = None

    def set_axon_ntff_profile_hook(h):
        mod._hook = h

    def get_axon_ntff_profile_hook():
        return mod._hook

    mod.set_axon_ntff_profile_hook = set_axon_ntff_profile_hook
    mod.get_axon_ntff_profile_hook = get_axon_ntff_profile_hook
    sys.modules["antenv.axon_hooks"] = mod
    antenv.axon_hooks = mod


_ensure_axon_hooks_module()

E_TOTAL = 2097152
C = 64
J_TOTAL = 524288
N_CORES = 8
PATTERN = (1, 3, 4, 8)  # segment lengths within one period
PERIOD_EDGES = 16
PERIOD_JUNCS = 4

E_LOC = E_TOTAL // N_CORES  # 262144 edges per core
J_LOC = J_TOTAL // N_CORES  # 65536 junctions per core
Q_LOC = J_LOC // PERIOD_JUNCS  # 16384 periods per core
R_OUT = 3  # device stores junctions 1..3 per period (junction 0 is host-filled)

# Periods-per-partition for each tile. A modest head tile starts compute a
# few us earlier; uniform big middle tiles keep the DVE op count low; the
# tapered tail shrinks the final (unoverlappable) output DMA. Sums to 128
# (= total periods per partition per core).
G_LIST = (8, 16, 16, 16, 16, 16, 16, 12, 8, 4)
G_MAX = max(G_LIST)

_COMPILED = None
LAST_RESULTS = None  # BassKernelResults of the most recent device run


def _build_program():
    import concourse.bacc as bacc
    import concourse.mybir as mybir
    from concourse.tile import TileContext

    MIN = mybir.AluOpType.min
    MAX = mybir.AluOpType.max
    BF16 = mybir.dt.bfloat16

    nc = bacc.Bacc()
    edges = nc.declare_dram_parameter("edges", [E_LOC, C], BF16, isOutput=False)
    out = nc.declare_dram_parameter(
        "out", [Q_LOC * R_OUT, 2 * C], BF16, isOutput=True
    )

    with TileContext(nc) as tc:
        with tc.tile_pool(name="iin", bufs=3) as pin, tc.tile_pool(
            name="iout", bufs=3
        ) as pout, tc.tile_pool(name="tmp", bufs=2) as tmp:
            edge_row = 0
            out_row = 0
            for g in G_LIST:
                # Per-tile views: partition p holds g whole 16-edge periods.
                n_in = 128 * g * PERIOD_EDGES
                n_out = 128 * g * R_OUT
                in_view = edges[edge_row : edge_row + n_in, :].rearrange(
                    "(p j) c -> p (j c)", p=128
                )
                out_view = out[out_row : out_row + n_out, :].rearrange(
                    "(p r) c -> p (r c)", p=128
                )
                edge_row += n_in
                out_row += n_out

                tile = pin.tile([128, g * PERIOD_EDGES * C], BF16, tag="itile")
                nc.sync.dma_start(out=tile[:], in_=in_view)
                otile = pout.tile([128, g * R_OUT * 2 * C], BF16, tag="otile")
                # v[p, g, x]: x = flat elements of one 16-edge period (1024)
                # edge e occupies x[e*64:(e+1)*64]; junction layout per period:
                # j0 = edge 0, j1 = edges 1..3, j2 = edges 4..7, j3 = 8..15.
                v = tile.rearrange("p (g x) -> p g x", g=g)
                # w[p, g, r, c]: junction r+1 of period g; c 0:64 = min,
                # 64:128 = max
                w = otile.rearrange(
                    "p (g r c) -> p g r c", g=g, r=R_OUT, c=2 * C
                )

                def tt(op, o, a, b):
                    nc.vector.tensor_tensor(out=o, in0=a, in1=b, op=op)

                for op, lo in ((MIN, 0), (MAX, C)):
                    sl = slice(lo, lo + C)
                    # Level 1, one op: pair edges (2k, 2k+1) for k=1..7 - every
                    # pair lies within a single segment (j1:{2,3}, j2:{4,5},
                    # {6,7}, j3:{8,9}..{14,15}).  m[k-1] = op(e_2k, e_2k+1).
                    tm = tmp.tile([128, g * 7 * C], BF16, tag="tm")
                    m = tm.rearrange("p (g x) -> p g x", g=g)
                    pairs = v[:, :, 2 * C : 16 * C].rearrange(
                        "p g (k c) -> p g k c", k=7
                    )
                    tt(op, m[:], pairs[:, :, :, 0:C], pairs[:, :, :, C : 2 * C])
                    # Level 2a, one op: pairs among m: (m1,m2)->j2, (m3,m4)->n0,
                    # (m5,m6)->n1   [q = [j2, n0, n1], contiguous]
                    tq = tmp.tile([128, g * 3 * C], BF16, tag="tq")
                    q = tq.rearrange("p (g x) -> p g x", g=g)
                    mp = m[:, :, C : 7 * C].rearrange("p g (k c) -> p g k c", k=3)
                    tt(op, q[:], mp[:, :, :, 0:C], mp[:, :, :, C : 2 * C])
                    # Level 2b: junction 1 (len-3) = op(m0, e1)
                    tt(op, w[:, :, 0, sl], m[:, :, 0:C], v[:, :, C : 2 * C])
                    # Level 3: junction 3 (len-8) = op(n0, n1)
                    tt(op, w[:, :, 2, sl], q[:, :, C : 2 * C], q[:, :, 2 * C : 3 * C])
                    # junction 2 (len-4) is q[0]; the idle scalar engine
                    # moves it into the output tile.
                    nc.scalar.copy(out=w[:, :, 1, sl], in_=q[:, :, 0:C])
                # Output DMAs go out via SWDGE (GpSimd): the SP HWDGE ring then
                # carries only input loads and never stalls behind an output's
                # wait-for-compute, so the input stream (which gates the DVE)
                # flows continuously.  (DVE here only runs tensor_tensor =
                # 2x_1P single-port mode, which never locks GpSimd out of the
                # shared SBUF port pair.)
                nc.gpsimd.dma_start(out=out_view, in_=otile[:])

    nc.compile()
    return nc


def _get_program():
    global _COMPILED
    if _COMPILED is None:
        _COMPILED = _build_program()
    return _COMPILED


def _to_bf16(x: np.ndarray):
    """f32 -> bf16 with round-to-nearest-even, as an ml_dtypes.bfloat16 view."""
    import ml_dtypes

    u = np.ascontiguousarray(x, dtype=np.float32).view(np.uint32)
    bias = ((u >> 16) & 1) + np.uint32(0x7FFF)
    return ((u + bias) >> 16).astype(np.uint16).view(ml_dtypes.bfloat16)


def _from_bf16(x: np.ndarray) -> np.ndarray:
    """bf16 (any 2-byte view) -> f32, exact."""
    u = np.ascontiguousarray(x).view(np.uint16).astype(np.uint32) << 16
    return u.view(np.float32)


def _pattern_matches(bounds: np.ndarray) -> bool:
    if bounds.shape != (J_TOTAL, 2):
        return False
    ends = bounds[:, 1].astype(np.int64)
    lengths = np.diff(ends, prepend=0)
    expect = np.tile(np.asarray(PATTERN, np.int64), J_TOTAL // PERIOD_JUNCS)
    return bool(ends[-1] == E_TOTAL and np.array_equal(lengths, expect))


def _fallback_host(edge_features: np.ndarray, bounds: np.ndarray) -> np.ndarray:
    # Generic reduction matching the reference's searchsorted-on-ends
    # semantics, including empty segments (+inf/-inf identities).
    ends = bounds[:, 1].astype(np.int64)
    J = bounds.shape[0]
    E = edge_features.shape[0]
    starts = np.concatenate([[0], ends[:-1]])
    starts = np.clip(starts, 0, E)
    ends_c = np.clip(ends, 0, E)
    mins = np.full((J, edge_features.shape[1]), np.inf, np.float32)
    maxs = np.full((J, edge_features.shape[1]), -np.inf, np.float32)
    for j in range(J):
        s, e = starts[j], ends_c[j]
        if e > s:
            seg = edge_features[s:e]
            mins[j] = seg.min(axis=0)
            maxs[j] = seg.max(axis=0)
    return np.concatenate([mins, maxs], axis=1)


def kernel(edge_features, cell_0_bounds) -> np.ndarray:
    global LAST_RESULTS
    edge_features = np.ascontiguousarray(np.asarray(edge_features, dtype=np.float32))
    cell_0_bounds = np.asarray(cell_0_bounds, dtype=np.int32)

    if edge_features.shape != (E_TOTAL, C) or not _pattern_matches(cell_0_bounds):
        return _fallback_host(edge_features, cell_0_bounds)

    from concourse.bass_utils import run_bass_kernel_spmd

    nc = _get_program()
    edges_bf16 = _to_bf16(edge_features)
    in_maps = [
        {"edges": edges_bf16[i * E_LOC : (i + 1) * E_LOC]} for i in range(N_CORES)
    ]
    res = run_bass_kernel_spmd(nc, in_maps, core_ids=list(range(N_CORES)))
    LAST_RESULTS = res

    full = np.empty((J_TOTAL, 2 * C), dtype=np.float32)
    # junction 0 of each period: min == max == edge 0 of the period, exact f32
    j0 = edge_features[0::PERIOD_EDGES]
    full[0::PERIOD_JUNCS, 0:C] = j0
    full[0::PERIOD_JUNCS, C:] = j0
    quads = full.reshape(J_TOTAL // PERIOD_JUNCS, PERIOD_JUNCS, 2 * C)
    for i, r in enumerate(res.results):
        dev = _from_bf16(r["out"]).reshape(Q_LOC, R_OUT, 2 * C)
        quads[i * Q_LOC : (i + 1) * Q_LOC, 1:, :] = dev
    return full
